# revision 21
# baseline (speedup 1.0000x reference)
"""2D DWT (db4, circular pad, stride-2) forward on 8 Trainium2 NeuronCores.

Final (pure data parallel, 12 images of 512x512 per core), fp16 datapath:
  stage 1 (filter along H):  V[w, (hj,a)]   = sum_h  X[h, w] * M[h, (hj,a)]
  stage 2 (filter along W):  out[hj,(wj,b)] = sum_w  V[w, a*256+hj] * M[w, (wj,b)]

Key mechanisms (each validated against NTFF traces of earlier versions):
- COMPACT banded M: each 128-row chunk keeps only its 134 nonzero columns.
- The whole per-core input is ONE dram tensor [128, 544 + 12*4096] fp16
  (M ++ images, contiguous per partition). The first DMA carries M + the
  first image pair in one 1.07 MB transfer (a small leading M DMA costs a
  ~2.8 us HWDGE completion bubble before pair0 otherwise); remaining
  pairs alternate between the SP and Act HWDGE rings (a single ring tops
  out ~250 GB/s; two overlap).
- PE warms up (HAM un-throttle needs ~3.4 us of activity) on a memset
  tile while the first input DMA is in flight.
- PSUM tiles span 2 banks (two 5-matmul accumulation groups per tile),
  drained by merged 1024-col copies with a 4-D de-interleaving access
  pattern: (hj,a)->(a,hj) on DVE for stage 1 (fp16 V feeds stage-2
  LDWEIGHTS contiguously), (wj,b)->(b,wj) on ScalarE for stage 2.
- PE program order is pipelined TWO images deep (stage 2 of image i-2
  after stage 1 of image i): DVE needs ~2.4 us to drain an image's V,
  longer than one stage-1 burst, so one-deep pipelining stalls ~1 us
  per step on the V-ready semaphore (measured via NTFF).

Output leaves as (img, hjc, p, (a, b, wj)) fp16; host unshuffles+upcasts.
"""

import sys

sys.path.insert(0, "/opt/trn_rl_repo")

import numpy as np

L = 512
NJ = L // 2  # 256
TAPS = 8
N_CORES = 8
IMGS_PER_CORE = 12  # 32 batch * 3 channels / 8 cores
MC = 136  # compact M columns per chunk (134 used + 2 pad)
XCOLS = 544 + IMGS_PER_CORE * 4 * L  # M ++ images, per-partition cols

_compiled = {}


def _build_M_compact(dec: np.ndarray) -> np.ndarray:
    """Compact banded filter matrix, chunk-major, partition-contiguous:
    Mc[p, c*136 + 2*t + f] = dec[f][(128c + p - 2*j_c(t)) mod 512] where
    j_c(t) = (64c - 3 + t) mod 256, t in [0, 67). Returns (128, 4*136)."""
    dec = np.asarray(dec, dtype=np.float32)
    Mc = np.zeros((128, 4 * MC), dtype=np.float32)
    p = np.arange(128)[:, None]
    t = np.arange(67)
    for c in range(4):
        j = (64 * c - 3 + t) % NJ
        k = (128 * c + p - 2 * j) % L
        mask = k < TAPS
        for f in range(2):
            Mc[:, c * MC + 2 * t + f] = np.where(
                mask, dec[f][np.minimum(k, TAPS - 1)], 0.0
            )
    return Mc


def _group_mms():
    """(chunk, rt_c0, rt_c1, out_c0, out_c1) slices for one accumulation
    group. Chunk c's compact cols [0,134) are j = 64c-3 .. 64c+63 (mod 256)
    interleaved; chunk 0 wraps: cols [0,6) -> out [506,512), [6,134) ->
    out [0,128). Order: big slices around the small wrap slice."""
    return [
        (1, 0, 134, 122, 256),
        (2, 0, 134, 250, 384),
        (0, 0, 6, 506, 512),
        (3, 0, 134, 378, 512),
        (0, 6, 134, 0, 128),
    ]


def _build_nc():
    import concourse.bass as bass  # noqa: F401
    import concourse.tile as tile
    from concourse import bacc, mybir

    f32 = mybir.dt.float32
    f16 = mybir.dt.float16
    nc = bacc.Bacc("TRN2", target_bir_lowering=False, debug=False,
                   num_devices=N_CORES)
    xall_d = nc.dram_tensor("xall", [128, XCOLS], f16, kind="ExternalInput")
    o_d = nc.dram_tensor("out", [IMGS_PER_CORE, 2, 128, 4 * NJ], f16,
                         kind="ExternalOutput")

    with tile.TileContext(nc) as tc:
        with (
            tc.tile_pool(name="mpool", bufs=1) as mpool,
            tc.tile_pool(name="xpool", bufs=5) as xpool,
            tc.tile_pool(name="vpool", bufs=4) as vpool,
            tc.tile_pool(name="opool", bufs=4) as opool,
            tc.tile_pool(name="pvpool", bufs=2, space="PSUM") as pvpool,
            tc.tile_pool(name="popool", bufs=2, space="PSUM") as popool,
        ):
            # PE warmup on a memset tile: no DMA dependency at all
            wt = mpool.tile([128, 128], f16, tag="wt")
            nc.gpsimd.memset(wt[:], 0.25)
            warm = popool.tile([128, 2 * L], f32, tag="po", name="warm")
            for _ in range(52):
                nc.tensor.matmul(warm[:, 0:128], wt[:], wt[:],
                                 start=True, stop=True)

            # first transfer: M ++ image pair 0 in one DMA on the SP ring
            big0 = mpool.tile([128, 544 + 2 * 4 * L], f16, tag="big0")
            nc.sync.dma_start(big0[:], xall_d[:, 0 : 544 + 8 * L])
            mth = big0[:, 0:544]

            # remaining pairs queued up front. Only the two EARLY-needed
            # pairs (1, 3) ride the Act ring: that ring is FIFO, and every
            # input byte queued there delays the first output DMA behind it
            xslc = {0: lambda m: big0[:, 544 + 4 * L * m : 544 + 4 * L * (m + 1)]}
            for pair in (1, 3, 2, 4, 5):
                xht = xpool.tile([128, 2 * 4 * L], f16, tag="xht")
                c0 = 544 + 8 * L * pair
                eng = nc.scalar if pair in (1, 3) else nc.sync
                eng.dma_start(xht[:], xall_d[:, c0 : c0 + 8 * L])
                xslc[pair] = (lambda t: lambda m: t[:, 4 * L * m : 4 * L * (m + 1)])(xht)

            vhts = {}
            for step in range(IMGS_PER_CORE + 2):
              if step < IMGS_PER_CORE:
                img = step
                xv = xslc[img // 2](img % 2)
                # stage 1: vht cols (wc, a, hj); lt = x chunk stationary
                vht = vpool.tile([128, 4 * L], f16, tag="vht")
                vhts[img] = vht
                for wcp in range(2):
                    pv = pvpool.tile([128, 2 * L], f32, tag="pv")
                    for half in range(2):
                        wc = 2 * wcp + half
                        o0 = L * half
                        for n, (hc, r0, r1, c0, c1) in enumerate(_group_mms()):
                            nc.tensor.matmul(
                                pv[:, o0 + c0 : o0 + c1],
                                xv[:, L * hc + 128 * wc : L * hc + 128 * wc + 128],
                                mth[:, MC * hc + r0 : MC * hc + r1],
                                start=(n == 0),
                                stop=(n == 4),
                            )
                    # merged drain: (wc, hj, a) -> (wc, a, hj), fp16
                    nc.vector.tensor_copy(
                        vht[:, 1024 * wcp : 1024 * wcp + 1024].rearrange(
                            "p (c a h) -> p c a h", c=2, a=2),
                        pv[:].rearrange("p (c h a) -> p c a h", c=2, a=2),
                    )
              if step >= 2:
                # stage 2 of image step-2: two full steps for DVE to finish
                # draining that image's V (one step is not enough; measured)
                img = step - 2
                vht = vhts.pop(img)
                ot = opool.tile([128, 2 * 4 * NJ], f16, tag="ot")
                for hjc in range(2):
                    po = popool.tile([128, 2 * L], f32, tag="po")
                    for a in range(2):
                        o0 = L * a
                        off = 256 * a + 128 * hjc
                        for n, (wc, r0, r1, c0, c1) in enumerate(_group_mms()):
                            nc.tensor.matmul(
                                po[:, o0 + c0 : o0 + c1],
                                vht[:, L * wc + off : L * wc + off + 128],
                                mth[:, MC * wc + r0 : MC * wc + r1],
                                start=(n == 0),
                                stop=(n == 4),
                            )
                    # merged drain: (a, wj, b) -> (a, b, wj), fp16
                    nc.scalar.copy(
                        ot[:, 1024 * hjc : 1024 * hjc + 1024].rearrange(
                            "p (c b w) -> p c b w", c=2, b=2),
                        po[:].rearrange("p (c w b) -> p c b w", c=2, b=2),
                    )
                # per-image output DMA on the Act HWDGE ring
                nc.scalar.dma_start(
                    o_d[img].rearrange("h p f -> p h f"),
                    ot[:].rearrange("p (h f) -> p h f", h=2),
                )

    nc.finalize()
    return nc


def _in_maps(x: np.ndarray, dec: np.ndarray) -> list[dict]:
    mh = _build_M_compact(dec).astype(np.float16)
    # (96, h, w) -> (96, p, c, w): partition p holds rows {p, 128+p, ...}
    xh = (x.reshape(96, 4, 128, L).swapaxes(1, 2)
           .astype(np.float16).reshape(96, 128, 4 * L))
    maps = []
    for c in range(N_CORES):
        xc = xh[IMGS_PER_CORE * c : IMGS_PER_CORE * (c + 1)]
        xall = np.concatenate(
            [mh, xc.transpose(1, 0, 2).reshape(128, IMGS_PER_CORE * 4 * L)],
            axis=1)
        maps.append({"xall": np.ascontiguousarray(xall)})
    return maps


def kernel(x: np.ndarray, dec: np.ndarray) -> np.ndarray:
    from concourse.bass_utils import run_bass_kernel_spmd

    x = np.ascontiguousarray(np.asarray(x, dtype=np.float32))
    dec = np.asarray(dec, dtype=np.float32)
    B, C, H, W = x.shape
    assert (B, C, H, W) == (32, 3, 512, 512) and dec.shape == (2, 8)

    if "nc" not in _compiled:
        _compiled["nc"] = _build_nc()
    nc = _compiled["nc"]

    in_maps = _in_maps(x, dec)
    res = run_bass_kernel_spmd(nc, in_maps, list(range(N_CORES))).results
    o = np.concatenate([r["out"] for r in res], axis=0)  # (96, 2, 128, 1024)
    # (img, hjc, p, a, b, wj) -> (img, s, hj, wj) with s = a + 2b
    o = o.reshape(96, 2, 128, 2, 2, NJ).transpose(0, 4, 3, 1, 2, 5)
    return np.ascontiguousarray(o, dtype=np.float32).reshape(B, C * 4, NJ, NJ)


# revision 22
# speedup vs baseline: 1.1273x; 1.1273x over previous
"""2D DWT (db4, circular pad, stride-2) forward on 8 Trainium2 NeuronCores.

Final (pure data parallel, 12 images of 512x512 per core), fp16 datapath:
  stage 1 (filter along H):  V[w, (hj,a)]   = sum_h  X[h, w] * M[h, (hj,a)]
  stage 2 (filter along W):  out[hj,(wj,b)] = sum_w  V[w, a*256+hj] * M[w, (wj,b)]

Key mechanisms (each validated against NTFF traces of earlier versions):
- COMPACT banded M: each 128-row chunk keeps only its 134 nonzero columns.
- The whole per-core input is ONE dram tensor [128, 544 + 12*4096] fp16
  (M ++ images, contiguous per partition). The first DMA carries M + the
  first image pair in one 1.07 MB transfer (a small leading M DMA costs a
  ~2.8 us HWDGE completion bubble before pair0 otherwise); remaining
  pairs alternate between the SP and Act HWDGE rings (a single ring tops
  out ~250 GB/s; two overlap).
- PE warms up (HAM un-throttle needs ~3.4 us of activity) on a memset
  tile while the first input DMA is in flight.
- PSUM tiles span 2 banks (two 5-matmul accumulation groups per tile),
  drained by merged 1024-col copies with a 4-D de-interleaving access
  pattern: (hj,a)->(a,hj) on DVE for stage 1 (fp16 V feeds stage-2
  LDWEIGHTS contiguously), (wj,b)->(b,wj) on ScalarE for stage 2.
- PE program order is pipelined TWO images deep (stage 2 of image i-2
  after stage 1 of image i): DVE needs ~2.4 us to drain an image's V,
  longer than one stage-1 burst, so one-deep pipelining stalls ~1 us
  per step on the V-ready semaphore (measured via NTFF).

Output leaves as (img, hjc, p, (a, b, wj)) fp16; host unshuffles+upcasts.
"""

import sys

sys.path.insert(0, "/opt/trn_rl_repo")

import numpy as np

L = 512
NJ = L // 2  # 256
TAPS = 8
N_CORES = 8
IMGS_PER_CORE = 12  # 32 batch * 3 channels / 8 cores
MC = 136  # compact M columns per chunk (134 used + 2 pad)
XCOLS = 544 + IMGS_PER_CORE * 4 * L  # M ++ images, per-partition cols

_compiled = {}


def _build_M_compact(dec: np.ndarray) -> np.ndarray:
    """Compact banded filter matrix, chunk-major, partition-contiguous:
    Mc[p, c*136 + 2*t + f] = dec[f][(128c + p - 2*j_c(t)) mod 512] where
    j_c(t) = (64c - 3 + t) mod 256, t in [0, 67). Returns (128, 4*136)."""
    dec = np.asarray(dec, dtype=np.float32)
    Mc = np.zeros((128, 4 * MC), dtype=np.float32)
    p = np.arange(128)[:, None]
    t = np.arange(67)
    for c in range(4):
        j = (64 * c - 3 + t) % NJ
        k = (128 * c + p - 2 * j) % L
        mask = k < TAPS
        for f in range(2):
            Mc[:, c * MC + 2 * t + f] = np.where(
                mask, dec[f][np.minimum(k, TAPS - 1)], 0.0
            )
    return Mc


def _group_mms():
    """(chunk, rt_c0, rt_c1, out_c0, out_c1) slices for one accumulation
    group. Chunk c's compact cols [0,134) are j = 64c-3 .. 64c+63 (mod 256)
    interleaved; chunk 0 wraps: cols [0,6) -> out [506,512), [6,134) ->
    out [0,128). Order: big slices around the small wrap slice."""
    return [
        (1, 0, 134, 122, 256),
        (2, 0, 134, 250, 384),
        (0, 0, 6, 506, 512),
        (3, 0, 134, 378, 512),
        (0, 6, 134, 0, 128),
    ]


def _build_nc():
    import concourse.bass as bass  # noqa: F401
    import concourse.tile as tile
    from concourse import bacc, mybir

    f32 = mybir.dt.float32
    f16 = mybir.dt.float16
    nc = bacc.Bacc("TRN2", target_bir_lowering=False, debug=False,
                   num_devices=N_CORES)
    xall_d = nc.dram_tensor("xall", [128, XCOLS], f16, kind="ExternalInput")
    o_d = nc.dram_tensor("out", [IMGS_PER_CORE, 2, 128, 4 * NJ], f16,
                         kind="ExternalOutput")

    with tile.TileContext(nc) as tc:
        with (
            tc.tile_pool(name="mpool", bufs=1) as mpool,
            tc.tile_pool(name="xpool", bufs=5) as xpool,
            tc.tile_pool(name="vpool", bufs=5) as vpool,
            tc.tile_pool(name="opool", bufs=4) as opool,
            tc.tile_pool(name="pvpool", bufs=2, space="PSUM") as pvpool,
            tc.tile_pool(name="popool", bufs=2, space="PSUM") as popool,
        ):
            # PE warmup on a memset tile: no DMA dependency at all
            wt = mpool.tile([128, 128], f16, tag="wt")
            nc.gpsimd.memset(wt[:], 0.25)
            warm = popool.tile([128, 2 * L], f32, tag="po", name="warm")
            for _ in range(52):
                nc.tensor.matmul(warm[:, 0:128], wt[:], wt[:],
                                 start=True, stop=True)

            # first transfer: M ++ image pair 0 in one DMA on the SP ring
            big0 = mpool.tile([128, 544 + 2 * 4 * L], f16, tag="big0")
            nc.sync.dma_start(big0[:], xall_d[:, 0 : 544 + 8 * L])
            mth = big0[:, 0:544]

            # remaining pairs queued up front. Only the two EARLY-needed
            # pairs (1, 3) ride the Act ring: that ring is FIFO, and every
            # input byte queued there delays the first output DMA behind it
            xslc = {0: lambda m: big0[:, 544 + 4 * L * m : 544 + 4 * L * (m + 1)]}
            for pair in (1, 3, 2, 4, 5):
                xht = xpool.tile([128, 2 * 4 * L], f16, tag="xht")
                c0 = 544 + 8 * L * pair
                eng = nc.scalar if pair in (1, 3) else nc.sync
                eng.dma_start(xht[:], xall_d[:, c0 : c0 + 8 * L])
                xslc[pair] = (lambda t: lambda m: t[:, 4 * L * m : 4 * L * (m + 1)])(xht)

            vhts = {}
            for step in range(IMGS_PER_CORE + 3):
              if step < IMGS_PER_CORE:
                img = step
                xv = xslc[img // 2](img % 2)
                # stage 1: vht cols (wc, a, hj); lt = x chunk stationary
                vht = vpool.tile([128, 4 * L], f16, tag="vht")
                vhts[img] = vht
                for wcp in range(2):
                    pv = pvpool.tile([128, 2 * L], f32, tag="pv")
                    for half in range(2):
                        wc = 2 * wcp + half
                        o0 = L * half
                        for n, (hc, r0, r1, c0, c1) in enumerate(_group_mms()):
                            nc.tensor.matmul(
                                pv[:, o0 + c0 : o0 + c1],
                                xv[:, L * hc + 128 * wc : L * hc + 128 * wc + 128],
                                mth[:, MC * hc + r0 : MC * hc + r1],
                                start=(n == 0),
                                stop=(n == 4),
                            )
                    # merged drain: (wc, hj, a) -> (wc, a, hj), fp16
                    nc.vector.tensor_copy(
                        vht[:, 1024 * wcp : 1024 * wcp + 1024].rearrange(
                            "p (c a h) -> p c a h", c=2, a=2),
                        pv[:].rearrange("p (c h a) -> p c a h", c=2, a=2),
                    )
              if step >= 3:
                # stage 2 of image step-3: DVE needs three stage-1 windows
                # to finish draining an image's V (S[161] stalls; measured)
                img = step - 3
                vht = vhts.pop(img)
                ot = opool.tile([128, 2 * 4 * NJ], f16, tag="ot")
                for hjc in range(2):
                    po = popool.tile([128, 2 * L], f32, tag="po")
                    for a in range(2):
                        o0 = L * a
                        off = 256 * a + 128 * hjc
                        for n, (wc, r0, r1, c0, c1) in enumerate(_group_mms()):
                            nc.tensor.matmul(
                                po[:, o0 + c0 : o0 + c1],
                                vht[:, L * wc + off : L * wc + off + 128],
                                mth[:, MC * wc + r0 : MC * wc + r1],
                                start=(n == 0),
                                stop=(n == 4),
                            )
                    # merged drain: (a, wj, b) -> (a, b, wj), fp16.
                    # In the stage-2-only tail DVE is idle: share drains.
                    drain = (nc.vector.tensor_copy
                             if img >= 9 and hjc == 0 else nc.scalar.copy)
                    drain(
                        ot[:, 1024 * hjc : 1024 * hjc + 1024].rearrange(
                            "p (c b w) -> p c b w", c=2, b=2),
                        po[:].rearrange("p (c w b) -> p c b w", c=2, b=2),
                    )
                # per-image output DMA on the Act HWDGE ring
                nc.scalar.dma_start(
                    o_d[img].rearrange("h p f -> p h f"),
                    ot[:].rearrange("p (h f) -> p h f", h=2),
                )

    nc.finalize()
    return nc


def _in_maps(x: np.ndarray, dec: np.ndarray) -> list[dict]:
    mh = _build_M_compact(dec).astype(np.float16)
    # (96, h, w) -> (96, p, c, w): partition p holds rows {p, 128+p, ...}
    xh = (x.reshape(96, 4, 128, L).swapaxes(1, 2)
           .astype(np.float16).reshape(96, 128, 4 * L))
    maps = []
    for c in range(N_CORES):
        xc = xh[IMGS_PER_CORE * c : IMGS_PER_CORE * (c + 1)]
        xall = np.concatenate(
            [mh, xc.transpose(1, 0, 2).reshape(128, IMGS_PER_CORE * 4 * L)],
            axis=1)
        maps.append({"xall": np.ascontiguousarray(xall)})
    return maps


def kernel(x: np.ndarray, dec: np.ndarray) -> np.ndarray:
    from concourse.bass_utils import run_bass_kernel_spmd

    x = np.ascontiguousarray(np.asarray(x, dtype=np.float32))
    dec = np.asarray(dec, dtype=np.float32)
    B, C, H, W = x.shape
    assert (B, C, H, W) == (32, 3, 512, 512) and dec.shape == (2, 8)

    if "nc" not in _compiled:
        _compiled["nc"] = _build_nc()
    nc = _compiled["nc"]

    in_maps = _in_maps(x, dec)
    res = run_bass_kernel_spmd(nc, in_maps, list(range(N_CORES))).results
    o = np.concatenate([r["out"] for r in res], axis=0)  # (96, 2, 128, 1024)
    # (img, hjc, p, a, b, wj) -> (img, s, hj, wj) with s = a + 2b
    o = o.reshape(96, 2, 128, 2, 2, NJ).transpose(0, 4, 3, 1, 2, 5)
    return np.ascontiguousarray(o, dtype=np.float32).reshape(B, C * 4, NJ, NJ)


# revision 23
# speedup vs baseline: 1.1732x; 1.0407x over previous
"""2D DWT (db4, circular pad, stride-2) forward on 8 Trainium2 NeuronCores.

Final (pure data parallel, 12 images of 512x512 per core), fp16 datapath:
  stage 1 (filter along H):  V[w, (hj,a)]   = sum_h  X[h, w] * M[h, (hj,a)]
  stage 2 (filter along W):  out[hj,(wj,b)] = sum_w  V[w, a*256+hj] * M[w, (wj,b)]

Key mechanisms (each validated against NTFF traces of earlier versions):
- COMPACT banded M: each 128-row chunk keeps only its 134 nonzero columns.
- The whole per-core input is ONE dram tensor [128, 544 + 12*4096] fp16
  (M ++ images, contiguous per partition). The first DMA carries M + the
  first image pair in one 1.07 MB transfer (a small leading M DMA costs a
  ~2.8 us HWDGE completion bubble before pair0 otherwise); remaining
  pairs alternate between the SP and Act HWDGE rings (a single ring tops
  out ~250 GB/s; two overlap).
- PE warms up (HAM un-throttle needs ~3.4 us of activity) on a memset
  tile while the first input DMA is in flight.
- PSUM tiles span 2 banks (two 5-matmul accumulation groups per tile),
  drained by merged 1024-col copies with a 4-D de-interleaving access
  pattern: (hj,a)->(a,hj) on DVE for stage 1 (fp16 V feeds stage-2
  LDWEIGHTS contiguously), (wj,b)->(b,wj) on ScalarE for stage 2.
- PE program order is pipelined TWO images deep (stage 2 of image i-2
  after stage 1 of image i): DVE needs ~2.4 us to drain an image's V,
  longer than one stage-1 burst, so one-deep pipelining stalls ~1 us
  per step on the V-ready semaphore (measured via NTFF).

Output leaves as (img, hjc, p, (a, b, wj)) fp16; host unshuffles+upcasts.
"""

import sys

sys.path.insert(0, "/opt/trn_rl_repo")

import numpy as np

L = 512
NJ = L // 2  # 256
TAPS = 8
N_CORES = 8
IMGS_PER_CORE = 12  # 32 batch * 3 channels / 8 cores
MC = 136  # compact M columns per chunk (134 used + 2 pad)
XCOLS = 544 + IMGS_PER_CORE * 4 * L  # M ++ images, per-partition cols

_compiled = {}


def _build_M_compact(dec: np.ndarray) -> np.ndarray:
    """Compact banded filter matrix, chunk-major, partition-contiguous:
    Mc[p, c*136 + 2*t + f] = dec[f][(128c + p - 2*j_c(t)) mod 512] where
    j_c(t) = (64c - 3 + t) mod 256, t in [0, 67). Returns (128, 4*136)."""
    dec = np.asarray(dec, dtype=np.float32)
    Mc = np.zeros((128, 4 * MC), dtype=np.float32)
    p = np.arange(128)[:, None]
    t = np.arange(67)
    for c in range(4):
        j = (64 * c - 3 + t) % NJ
        k = (128 * c + p - 2 * j) % L
        mask = k < TAPS
        for f in range(2):
            Mc[:, c * MC + 2 * t + f] = np.where(
                mask, dec[f][np.minimum(k, TAPS - 1)], 0.0
            )
    return Mc


def _group_mms():
    """(chunk, rt_c0, rt_c1, out_c0, out_c1) slices for one accumulation
    group. Chunk c's compact cols [0,134) are j = 64c-3 .. 64c+63 (mod 256)
    interleaved; chunk 0 wraps: cols [0,6) -> out [506,512), [6,134) ->
    out [0,128). Order: big slices around the small wrap slice."""
    return [
        (1, 0, 134, 122, 256),
        (2, 0, 134, 250, 384),
        (0, 0, 6, 506, 512),
        (3, 0, 134, 378, 512),
        (0, 6, 134, 0, 128),
    ]


def _build_nc():
    import concourse.bass as bass  # noqa: F401
    import concourse.tile as tile
    from concourse import bacc, mybir

    f32 = mybir.dt.float32
    f16 = mybir.dt.float16
    nc = bacc.Bacc("TRN2", target_bir_lowering=False, debug=False,
                   num_devices=N_CORES)
    xall_d = nc.dram_tensor("xall", [128, XCOLS], f16, kind="ExternalInput")
    o_d = nc.dram_tensor("out", [IMGS_PER_CORE, 2, 128, 4 * NJ], f16,
                         kind="ExternalOutput")

    with tile.TileContext(nc) as tc:
        with (
            tc.tile_pool(name="mpool", bufs=1) as mpool,
            tc.tile_pool(name="xpool", bufs=5) as xpool,
            tc.tile_pool(name="vpool", bufs=6) as vpool,
            tc.tile_pool(name="opool", bufs=4) as opool,
            tc.tile_pool(name="pvpool", bufs=2, space="PSUM") as pvpool,
            tc.tile_pool(name="popool", bufs=2, space="PSUM") as popool,
        ):
            # PE warmup on a memset tile: no DMA dependency at all
            wt = mpool.tile([128, 128], f16, tag="wt")
            nc.gpsimd.memset(wt[:], 0.25)
            warm = popool.tile([128, 2 * L], f32, tag="po", name="warm")
            for _ in range(52):
                nc.tensor.matmul(warm[:, 0:128], wt[:], wt[:],
                                 start=True, stop=True)

            # first transfer: M ++ image pair 0 in one DMA on the SP ring
            big0 = mpool.tile([128, 544 + 2 * 4 * L], f16, tag="big0")
            nc.sync.dma_start(big0[:], xall_d[:, 0 : 544 + 8 * L])
            mth = big0[:, 0:544]

            # remaining pairs queued up front. Only the two EARLY-needed
            # pairs (1, 3) ride the Act ring: that ring is FIFO, and every
            # input byte queued there delays the first output DMA behind it
            xslc = {0: lambda m: big0[:, 544 + 4 * L * m : 544 + 4 * L * (m + 1)]}
            for pair in (1, 3, 2, 4, 5):
                xht = xpool.tile([128, 2 * 4 * L], f16, tag="xht")
                c0 = 544 + 8 * L * pair
                eng = nc.scalar if pair in (1, 3) else nc.sync
                eng.dma_start(xht[:], xall_d[:, c0 : c0 + 8 * L])
                xslc[pair] = (lambda t: lambda m: t[:, 4 * L * m : 4 * L * (m + 1)])(xht)

            vhts = {}
            for step in range(IMGS_PER_CORE + 4):
              if step < IMGS_PER_CORE:
                img = step
                xv = xslc[img // 2](img % 2)
                # stage 1: vht cols (wc, a, hj); lt = x chunk stationary
                vht = vpool.tile([128, 4 * L], f16, tag="vht")
                vhts[img] = vht
                for wcp in range(2):
                    pv = pvpool.tile([128, 2 * L], f32, tag="pv")
                    for half in range(2):
                        wc = 2 * wcp + half
                        o0 = L * half
                        for n, (hc, r0, r1, c0, c1) in enumerate(_group_mms()):
                            nc.tensor.matmul(
                                pv[:, o0 + c0 : o0 + c1],
                                xv[:, L * hc + 128 * wc : L * hc + 128 * wc + 128],
                                mth[:, MC * hc + r0 : MC * hc + r1],
                                start=(n == 0),
                                stop=(n == 4),
                            )
                    # merged drain: (wc, hj, a) -> (wc, a, hj), fp16
                    nc.vector.tensor_copy(
                        vht[:, 1024 * wcp : 1024 * wcp + 1024].rearrange(
                            "p (c a h) -> p c a h", c=2, a=2),
                        pv[:].rearrange("p (c h a) -> p c a h", c=2, a=2),
                    )
              if step >= 4:
                # stage 2 of image step-4: DVE backlog means an image's V
                # drains finish ~3 stage-1 windows late (S[161]; measured)
                img = step - 4
                vht = vhts.pop(img)
                ot = opool.tile([128, 2 * 4 * NJ], f16, tag="ot")
                for hjc in range(2):
                    po = popool.tile([128, 2 * L], f32, tag="po")
                    for a in range(2):
                        o0 = L * a
                        off = 256 * a + 128 * hjc
                        for n, (wc, r0, r1, c0, c1) in enumerate(_group_mms()):
                            nc.tensor.matmul(
                                po[:, o0 + c0 : o0 + c1],
                                vht[:, L * wc + off : L * wc + off + 128],
                                mth[:, MC * wc + r0 : MC * wc + r1],
                                start=(n == 0),
                                stop=(n == 4),
                            )
                    # merged drain: (a, wj, b) -> (a, b, wj), fp16.
                    # In the stage-2-only tail DVE is idle: share drains.
                    drain = (nc.vector.tensor_copy
                             if img >= 8 and hjc == 0 else nc.scalar.copy)
                    drain(
                        ot[:, 1024 * hjc : 1024 * hjc + 1024].rearrange(
                            "p (c b w) -> p c b w", c=2, b=2),
                        po[:].rearrange("p (c w b) -> p c b w", c=2, b=2),
                    )
                # per-image output DMA via GpSimd SWDGE: its own
                # descriptor path + queue, and Act sheds the issue cost
                nc.gpsimd.dma_start(
                    o_d[img].rearrange("h p f -> p h f"),
                    ot[:].rearrange("p (h f) -> p h f", h=2),
                )

    nc.finalize()
    return nc


def _in_maps(x: np.ndarray, dec: np.ndarray) -> list[dict]:
    mh = _build_M_compact(dec).astype(np.float16)
    # (96, h, w) -> (96, p, c, w): partition p holds rows {p, 128+p, ...}
    xh = (x.reshape(96, 4, 128, L).swapaxes(1, 2)
           .astype(np.float16).reshape(96, 128, 4 * L))
    maps = []
    for c in range(N_CORES):
        xc = xh[IMGS_PER_CORE * c : IMGS_PER_CORE * (c + 1)]
        xall = np.concatenate(
            [mh, xc.transpose(1, 0, 2).reshape(128, IMGS_PER_CORE * 4 * L)],
            axis=1)
        maps.append({"xall": np.ascontiguousarray(xall)})
    return maps


def kernel(x: np.ndarray, dec: np.ndarray) -> np.ndarray:
    from concourse.bass_utils import run_bass_kernel_spmd

    x = np.ascontiguousarray(np.asarray(x, dtype=np.float32))
    dec = np.asarray(dec, dtype=np.float32)
    B, C, H, W = x.shape
    assert (B, C, H, W) == (32, 3, 512, 512) and dec.shape == (2, 8)

    if "nc" not in _compiled:
        _compiled["nc"] = _build_nc()
    nc = _compiled["nc"]

    in_maps = _in_maps(x, dec)
    res = run_bass_kernel_spmd(nc, in_maps, list(range(N_CORES))).results
    o = np.concatenate([r["out"] for r in res], axis=0)  # (96, 2, 128, 1024)
    # (img, hjc, p, a, b, wj) -> (img, s, hj, wj) with s = a + 2b
    o = o.reshape(96, 2, 128, 2, 2, NJ).transpose(0, 4, 3, 1, 2, 5)
    return np.ascontiguousarray(o, dtype=np.float32).reshape(B, C * 4, NJ, NJ)


# revision 24
# speedup vs baseline: 1.2493x; 1.0648x over previous
"""2D DWT (db4, circular pad, stride-2) forward on 8 Trainium2 NeuronCores.

Final (pure data parallel, 12 images of 512x512 per core), fp16 datapath:
  stage 1 (filter along H):  V[w, (hj,a)]   = sum_h  X[h, w] * M[h, (hj,a)]
  stage 2 (filter along W):  out[hj,(wj,b)] = sum_w  V[w, a*256+hj] * M[w, (wj,b)]

Key mechanisms (each validated against NTFF traces of earlier versions):
- COMPACT banded M: each 128-row chunk keeps only its 134 nonzero columns.
- The whole per-core input is ONE dram tensor [128, 544 + 12*4096] fp16
  (M ++ images, contiguous per partition). The first DMA carries M + the
  first image pair in one 1.07 MB transfer (a small leading M DMA costs a
  ~2.8 us HWDGE completion bubble before pair0 otherwise); remaining
  pairs alternate between the SP and Act HWDGE rings (a single ring tops
  out ~250 GB/s; two overlap).
- PE warms up (HAM un-throttle needs ~3.4 us of activity) on a memset
  tile while the first input DMA is in flight.
- PSUM tiles span 2 banks (two 5-matmul accumulation groups per tile),
  drained by merged 1024-col copies with a 4-D de-interleaving access
  pattern: (hj,a)->(a,hj) on DVE for stage 1 (fp16 V feeds stage-2
  LDWEIGHTS contiguously), (wj,b)->(b,wj) on ScalarE for stage 2.
- PE program order is pipelined TWO images deep (stage 2 of image i-2
  after stage 1 of image i): DVE needs ~2.4 us to drain an image's V,
  longer than one stage-1 burst, so one-deep pipelining stalls ~1 us
  per step on the V-ready semaphore (measured via NTFF).

Output leaves as (img, hjc, p, (a, b, wj)) fp16; host unshuffles+upcasts.
"""

import sys

sys.path.insert(0, "/opt/trn_rl_repo")

import numpy as np

L = 512
NJ = L // 2  # 256
TAPS = 8
N_CORES = 8
IMGS_PER_CORE = 12  # 32 batch * 3 channels / 8 cores
MC = 136  # compact M columns per chunk (134 used + 2 pad)
XCOLS = 544 + IMGS_PER_CORE * 4 * L  # M ++ images, per-partition cols

_compiled = {}


def _build_M_compact(dec: np.ndarray) -> np.ndarray:
    """Compact banded filter matrix, chunk-major, partition-contiguous:
    Mc[p, c*136 + 2*t + f] = dec[f][(128c + p - 2*j_c(t)) mod 512] where
    j_c(t) = (64c - 3 + t) mod 256, t in [0, 67). Returns (128, 4*136)."""
    dec = np.asarray(dec, dtype=np.float32)
    Mc = np.zeros((128, 4 * MC), dtype=np.float32)
    p = np.arange(128)[:, None]
    t = np.arange(67)
    for c in range(4):
        j = (64 * c - 3 + t) % NJ
        k = (128 * c + p - 2 * j) % L
        mask = k < TAPS
        for f in range(2):
            Mc[:, c * MC + 2 * t + f] = np.where(
                mask, dec[f][np.minimum(k, TAPS - 1)], 0.0
            )
    return Mc


def _group_mms():
    """(chunk, rt_c0, rt_c1, out_c0, out_c1) slices for one accumulation
    group. Chunk c's compact cols [0,134) are j = 64c-3 .. 64c+63 (mod 256)
    interleaved; chunk 0 wraps: cols [0,6) -> out [506,512), [6,134) ->
    out [0,128). Order: big slices around the small wrap slice."""
    return [
        (1, 0, 134, 122, 256),
        (2, 0, 134, 250, 384),
        (0, 0, 6, 506, 512),
        (3, 0, 134, 378, 512),
        (0, 6, 134, 0, 128),
    ]


def _build_nc():
    import concourse.bass as bass  # noqa: F401
    import concourse.tile as tile
    from concourse import bacc, mybir

    f32 = mybir.dt.float32
    f16 = mybir.dt.float16
    nc = bacc.Bacc("TRN2", target_bir_lowering=False, debug=False,
                   num_devices=N_CORES)
    xall_d = nc.dram_tensor("xall", [128, XCOLS], f16, kind="ExternalInput")
    o_d = nc.dram_tensor("out", [IMGS_PER_CORE, 2, 128, 4 * NJ], f16,
                         kind="ExternalOutput")

    with tile.TileContext(nc) as tc:
        with (
            tc.tile_pool(name="mpool", bufs=1) as mpool,
            tc.tile_pool(name="xpool", bufs=5) as xpool,
            tc.tile_pool(name="vpool", bufs=6) as vpool,
            tc.tile_pool(name="opool", bufs=4) as opool,
            tc.tile_pool(name="pvpool", bufs=2, space="PSUM") as pvpool,
            tc.tile_pool(name="popool", bufs=2, space="PSUM") as popool,
        ):
            # PE warmup on a memset tile: no DMA dependency at all
            wt = mpool.tile([128, 128], f16, tag="wt")
            nc.gpsimd.memset(wt[:], 0.25)
            warm = popool.tile([128, 2 * L], f32, tag="po", name="warm")
            for _ in range(52):
                nc.tensor.matmul(warm[:, 0:128], wt[:], wt[:],
                                 start=True, stop=True)

            # first transfer: M ++ image pair 0 in one DMA on the SP ring
            big0 = mpool.tile([128, 544 + 2 * 4 * L], f16, tag="big0")
            nc.sync.dma_start(big0[:], xall_d[:, 0 : 544 + 8 * L])
            mth = big0[:, 0:544]

            # remaining pairs queued up front. Only the two EARLY-needed
            # pairs (1, 3) ride the Act ring: that ring is FIFO, and every
            # input byte queued there delays the first output DMA behind it
            xslc = {0: lambda m: big0[:, 544 + 4 * L * m : 544 + 4 * L * (m + 1)]}
            for pair in (1, 3, 2, 4, 5):
                xht = xpool.tile([128, 2 * 4 * L], f16, tag="xht")
                c0 = 544 + 8 * L * pair
                eng = nc.scalar if pair in (1, 3) else nc.sync
                eng.dma_start(xht[:], xall_d[:, c0 : c0 + 8 * L])
                xslc[pair] = (lambda t: lambda m: t[:, 4 * L * m : 4 * L * (m + 1)])(xht)

            vhts = {}
            for step in range(IMGS_PER_CORE + 4):
              if step < IMGS_PER_CORE:
                img = step
                xv = xslc[img // 2](img % 2)
                # stage 1: vht cols (wc, a, hj); lt = x chunk stationary
                vht = vpool.tile([128, 4 * L], f16, tag="vht")
                vhts[img] = vht
                for wcp in range(2):
                    pv = pvpool.tile([128, 2 * L], f32, tag="pv")
                    for half in range(2):
                        wc = 2 * wcp + half
                        o0 = L * half
                        for n, (hc, r0, r1, c0, c1) in enumerate(_group_mms()):
                            nc.tensor.matmul(
                                pv[:, o0 + c0 : o0 + c1],
                                xv[:, L * hc + 128 * wc : L * hc + 128 * wc + 128],
                                mth[:, MC * hc + r0 : MC * hc + r1],
                                start=(n == 0),
                                stop=(n == 4),
                            )
                    # merged drain: (wc, hj, a) -> (wc, a, hj), fp16.
                    # The two drains go to DIFFERENT engines so they run
                    # in parallel and V is ready a full drain earlier.
                    d1 = nc.vector.tensor_copy if wcp == 0 else nc.scalar.copy
                    d1(
                        vht[:, 1024 * wcp : 1024 * wcp + 1024].rearrange(
                            "p (c a h) -> p c a h", c=2, a=2),
                        pv[:].rearrange("p (c h a) -> p c a h", c=2, a=2),
                    )
              if step >= 4:
                # stage 2 of image step-4: DVE backlog means an image's V
                # drains finish ~3 stage-1 windows late (S[161]; measured)
                img = step - 4
                vht = vhts.pop(img)
                ot = opool.tile([128, 2 * 4 * NJ], f16, tag="ot")
                for hjc in range(2):
                    po = popool.tile([128, 2 * L], f32, tag="po")
                    for a in range(2):
                        o0 = L * a
                        off = 256 * a + 128 * hjc
                        for n, (wc, r0, r1, c0, c1) in enumerate(_group_mms()):
                            nc.tensor.matmul(
                                po[:, o0 + c0 : o0 + c1],
                                vht[:, L * wc + off : L * wc + off + 128],
                                mth[:, MC * wc + r0 : MC * wc + r1],
                                start=(n == 0),
                                stop=(n == 4),
                            )
                    # merged drain: (a, wj, b) -> (a, b, wj), fp16;
                    # split across engines like stage 1 (balanced loads)
                    d2 = nc.vector.tensor_copy if hjc == 0 else nc.scalar.copy
                    d2(
                        ot[:, 1024 * hjc : 1024 * hjc + 1024].rearrange(
                            "p (c b w) -> p c b w", c=2, b=2),
                        po[:].rearrange("p (c w b) -> p c b w", c=2, b=2),
                    )
                # per-image output DMA via GpSimd SWDGE: its own
                # descriptor path + queue, and Act sheds the issue cost
                nc.gpsimd.dma_start(
                    o_d[img].rearrange("h p f -> p h f"),
                    ot[:].rearrange("p (h f) -> p h f", h=2),
                )

    nc.finalize()
    return nc


def _in_maps(x: np.ndarray, dec: np.ndarray) -> list[dict]:
    mh = _build_M_compact(dec).astype(np.float16)
    # (96, h, w) -> (96, p, c, w): partition p holds rows {p, 128+p, ...}
    xh = (x.reshape(96, 4, 128, L).swapaxes(1, 2)
           .astype(np.float16).reshape(96, 128, 4 * L))
    maps = []
    for c in range(N_CORES):
        xc = xh[IMGS_PER_CORE * c : IMGS_PER_CORE * (c + 1)]
        xall = np.concatenate(
            [mh, xc.transpose(1, 0, 2).reshape(128, IMGS_PER_CORE * 4 * L)],
            axis=1)
        maps.append({"xall": np.ascontiguousarray(xall)})
    return maps


def kernel(x: np.ndarray, dec: np.ndarray) -> np.ndarray:
    from concourse.bass_utils import run_bass_kernel_spmd

    x = np.ascontiguousarray(np.asarray(x, dtype=np.float32))
    dec = np.asarray(dec, dtype=np.float32)
    B, C, H, W = x.shape
    assert (B, C, H, W) == (32, 3, 512, 512) and dec.shape == (2, 8)

    if "nc" not in _compiled:
        _compiled["nc"] = _build_nc()
    nc = _compiled["nc"]

    in_maps = _in_maps(x, dec)
    res = run_bass_kernel_spmd(nc, in_maps, list(range(N_CORES))).results
    o = np.concatenate([r["out"] for r in res], axis=0)  # (96, 2, 128, 1024)
    # (img, hjc, p, a, b, wj) -> (img, s, hj, wj) with s = a + 2b
    o = o.reshape(96, 2, 128, 2, 2, NJ).transpose(0, 4, 3, 1, 2, 5)
    return np.ascontiguousarray(o, dtype=np.float32).reshape(B, C * 4, NJ, NJ)
